# revision 1
# baseline (speedup 1.0000x reference)
"""Trainium2 Bass kernel for nn_LQE (topk_masking).

out = scores + MLP(topk_softmax_stats(pred_corners))

Math notes:
- top_k(softmax(x)) == softmax values of top_k(x) (softmax is monotone), and
  exp(x) is safe unnormalized here (|x| <~ 6 for randn inputs), so
  p_k = exp(x)_k / sum(exp(x)).  DVE `max` returns top-8 sorted descending in
  one instruction -> top-4 directly.
- The mean-of-top4 stat feature folds into W1 on the host:
  W1eff[c*4+k] = W1[c*5+k] + 0.25*W1[c*5+4]; b2 folds into scores.

Sharding: pure data-parallel over B*L = 320000 rows, 40000 rows/core on 8
cores, padded to 40960 = 40 supertiles x 1024 rows.  Within a supertile,
row = 8*p + g for partition p in [0,128), group g in [0,8).
"""

import numpy as np

K = 4
C = 4
NB = 33
HID = 64
G = 8  # row-groups per partition per supertile
ROWS_PER_TILE = 128 * G  # 1024
N_CORES = 8
ROWS_PER_CORE = 40000
ROWS_PAD = 40960  # 40 supertiles
N_TILES = ROWS_PAD // ROWS_PER_TILE

_CACHE = {}


def _build(rows):
    import concourse.bacc as bacc
    import concourse.mybir as mybir
    from concourse.tile import TileContext

    f32 = mybir.dt.float32
    n_tiles = rows // ROWS_PER_TILE
    assert rows % ROWS_PER_TILE == 0

    nc = bacc.Bacc("TRN2")
    pred = nc.dram_tensor("pred", [rows, C * NB], f32, kind="ExternalInput")
    scores = nc.dram_tensor("scores", [rows, 80], f32, kind="ExternalInput")
    w1 = nc.dram_tensor("w1", [64, 256], f32, kind="ExternalInput")
    w2 = nc.dram_tensor("w2", [64, 64], f32, kind="ExternalInput")
    b1 = nc.dram_tensor("b1", [64, 1], f32, kind="ExternalInput")
    ident = nc.dram_tensor("ident", [128, 128], f32, kind="ExternalInput")
    out = nc.dram_tensor("out", [rows, 80], f32, kind="ExternalOutput")

    NG = G * C  # 32 softmax groups per partition
    with TileContext(nc) as tc:
        with (
            tc.tile_pool(name="singles", bufs=1) as singles,
            tc.tile_pool(name="pin", bufs=3) as pin,
            tc.tile_pool(name="sin", bufs=3) as sin,
            tc.tile_pool(name="epool", bufs=3) as epool,
            tc.tile_pool(name="tpool", bufs=3) as tpool,
            tc.tile_pool(name="cpool", bufs=3) as cpool,
            tc.tile_pool(name="small", bufs=4) as small,
            tc.tile_pool(name="statp", bufs=3) as statp,
            tc.tile_pool(name="stp", bufs=3) as stp,
            tc.tile_pool(name="hpool", bufs=3) as hpool,
            tc.tile_pool(name="qpool", bufs=3) as qpool,
            tc.tile_pool(name="opool", bufs=3) as opool,
            tc.tile_pool(name="ps_t", bufs=2, space="PSUM") as ps_t,
            tc.tile_pool(name="ps_h", bufs=2, space="PSUM") as ps_h,
            tc.tile_pool(name="ps_q", bufs=2, space="PSUM") as ps_q,
        ):
            w1_sb = singles.tile([64, 256], f32)
            nc.sync.dma_start(out=w1_sb, in_=w1[:, :])
            w2_sb = singles.tile([64, 64], f32)
            nc.sync.dma_start(out=w2_sb, in_=w2[:, :])
            b1_sb = singles.tile([64, 1], f32)
            nc.sync.dma_start(out=b1_sb, in_=b1[:, :])
            ident_sb = singles.tile([128, 128], f32)
            nc.sync.dma_start(out=ident_sb, in_=ident[:, :])

            for s in range(n_tiles):
                r0 = s * ROWS_PER_TILE
                # ---- loads (row = G*p + g) ----
                pred_t = pin.tile([128, G, C * NB], f32)
                nc.sync.dma_start(
                    out=pred_t,
                    in_=pred[r0 : r0 + ROWS_PER_TILE, :].rearrange(
                        "(p g) d -> p g d", p=128
                    ),
                )
                scores_t = sin.tile([128, G, 80], f32)
                nc.sync.dma_start(
                    out=scores_t,
                    in_=scores[r0 : r0 + ROWS_PER_TILE, :].rearrange(
                        "(p g) d -> p g d", p=128
                    ),
                )

                # ---- e = exp(pred) on ACT ----
                e = epool.tile([128, G, C * NB], f32)
                nc.scalar.activation(
                    out=e, in_=pred_t, func=mybir.ActivationFunctionType.Exp
                )
                ef = e.rearrange("p g d -> p (g d)")

                # ---- top-8 per softmax group on DVE (sorted desc) ----
                t8 = tpool.tile([128, NG * 8], f32)
                for i in range(NG):
                    nc.vector.max(
                        out=t8[:, i * 8 : i * 8 + 8],
                        in_=ef[:, i * NB : (i + 1) * NB],
                    )

                # ---- denominators: first reduction level on GPSIMD ----
                ef3 = e.rearrange("p g (c b) -> p (g c) b", b=NB)
                c16 = cpool.tile([128, NG, 16], f32)
                nc.gpsimd.tensor_tensor(
                    out=c16,
                    in0=ef3[:, :, 0:16],
                    in1=ef3[:, :, 16:32],
                    op=mybir.AluOpType.add,
                )
                s_part = small.tile([128, NG], f32)
                nc.vector.tensor_reduce(
                    out=s_part,
                    in_=c16,
                    axis=mybir.AxisListType.X,
                    op=mybir.AluOpType.add,
                )
                s32 = small.tile([128, NG], f32)
                nc.vector.tensor_add(s32, s_part, ef3[:, :, 32])
                r32 = small.tile([128, NG], f32)
                nc.vector.reciprocal(out=r32, in_=s32)

                # ---- stat = top4 * (1/s) on GPSIMD ----
                stat = statp.tile([128, NG, K], f32)
                nc.gpsimd.tensor_tensor(
                    out=stat,
                    in0=t8.rearrange("p (i k) -> p i k", k=8)[:, :, 0:K],
                    in1=r32[:].unsqueeze(2).broadcast_to([128, NG, K]),
                    op=mybir.AluOpType.mult,
                )

                # ---- statT = stat.T via PE (two 64-col halves); copy to SBUF ----
                statf = stat.rearrange("p i k -> p (i k)")
                statT = ps_t.tile([64, 256], f32)
                nc.tensor.transpose(
                    out=statT[:, 0:128], in_=statf[:, 0:64], identity=ident_sb
                )
                nc.tensor.transpose(
                    out=statT[:, 128:256], in_=statf[:, 64:128], identity=ident_sb
                )
                statT_sb = stp.tile([64, 256], f32)
                nc.scalar.copy(out=statT_sb, in_=statT)

                # ---- MLP layer 1: per-group zero-padded W1 slices, all base-0 ----
                # statT[c, half*128+p] = stat[p, half*64+c]; group g = half*4+gp,
                # c = gp*16+f.  lhsT = w1sel_gp [64,64] (rows gp*16..+16 = W1eff,
                # rest zero) so the K=64 contraction selects group gp only.
                # hT col layout: g*128 + p.
                hT = ps_h.tile([HID, G * 128], f32)
                for half in range(2):
                    for gp in range(4):
                        g = half * 4 + gp
                        nc.tensor.matmul(
                            out=hT[:, g * 128 : (g + 1) * 128],
                            lhsT=w1_sb[:, gp * HID : (gp + 1) * HID],
                            rhs=statT_sb[:, half * 128 : (half + 1) * 128],
                            start=True,
                            stop=True,
                        )
                hT_sb = hpool.tile([HID, G * 128], f32)
                nc.scalar.activation(
                    out=hT_sb,
                    in_=hT,
                    func=mybir.ActivationFunctionType.Relu,
                    bias=b1_sb,
                    scale=1.0,
                )

                # ---- MLP layer 2: one-hot W2 accumulation, bank-aligned out ----
                qcol = ps_q.tile([128, G], f32)
                for g in range(G):
                    nc.tensor.matmul(
                        out=qcol,
                        lhsT=hT_sb[:, g * 128 : (g + 1) * 128],
                        rhs=w2_sb[:, g * G : (g + 1) * G],
                        start=(g == 0),
                        stop=(g == G - 1),
                    )
                q_sb = qpool.tile([128, G], f32)
                nc.scalar.copy(out=q_sb, in_=qcol)

                # ---- out = scores + q (broadcast over 80) on GPSIMD ----
                out_t = opool.tile([128, G, 80], f32)
                nc.gpsimd.tensor_tensor(
                    out=out_t,
                    in0=scores_t,
                    in1=q_sb[:].unsqueeze(2).broadcast_to([128, G, 80]),
                    op=mybir.AluOpType.add,
                )
                nc.sync.dma_start(
                    out=out[r0 : r0 + ROWS_PER_TILE, :].rearrange(
                        "(p g) d -> p g d", p=128
                    ),
                    in_=out_t,
                )
    nc.compile()
    return nc


def _get_nc(rows):
    if rows not in _CACHE:
        _CACHE[rows] = _build(rows)
    return _CACHE[rows]


def _prep_host(scores, pred_corners, W1, b1, W2, b2):
    B, L, c, nb = pred_corners.shape
    BL = B * L
    scores_f = np.ascontiguousarray(
        scores.reshape(BL, scores.shape[-1]), dtype=np.float32
    ) + np.float32(b2[0])
    pred_f = np.ascontiguousarray(pred_corners.reshape(BL, c * nb), dtype=np.float32)
    idx = [ci * (K + 1) + k for ci in range(C) for k in range(K)]
    W1eff = (W1[idx] + 0.25 * np.repeat(W1[K :: K + 1], K, axis=0)).astype(np.float32)
    # w1sel: per-subgroup zero-padded copies of W1eff (K=64 contraction
    # selects subgroup gp via zero rows)
    w1sel = np.zeros((64, 256), np.float32)
    for gp in range(4):
        w1sel[gp * 16 : (gp + 1) * 16, gp * HID : (gp + 1) * HID] = W1eff
    # w2oh: one-hot column copies of W2 for the mm2 accumulation trick
    w2oh = np.zeros((HID, 64), np.float32)
    for g in range(8):
        w2oh[:, g * 8 + g] = W2[:, 0].astype(np.float32)
    b1c = np.ascontiguousarray(b1.astype(np.float32).reshape(HID, 1))
    ident = np.eye(128, dtype=np.float32)
    return scores_f, pred_f, w1sel, w2oh, b1c, ident


def _run(scores, pred_corners, W1, b1, W2, b2, trace=False):
    from concourse.bass_utils import run_bass_kernel_spmd

    B, L, _, _ = pred_corners.shape
    BL = B * L
    scores_f, pred_f, w1sel, w2oh, b1c, ident = _prep_host(
        scores, pred_corners, W1, b1, W2, b2
    )
    nc = _get_nc(ROWS_PAD)
    in_maps = []
    for i in range(N_CORES):
        lo = i * ROWS_PER_CORE
        hi = lo + ROWS_PER_CORE
        npad = ROWS_PAD - ROWS_PER_CORE
        p_i = np.concatenate(
            [pred_f[lo:hi], np.zeros((npad, C * NB), np.float32)], axis=0
        )
        s_i = np.concatenate([scores_f[lo:hi], np.zeros((npad, 80), np.float32)], axis=0)
        in_maps.append(
            {
                "pred": p_i,
                "scores": s_i,
                "w1": w1sel,
                "w2": w2oh,
                "b1": b1c,
                "ident": ident,
            }
        )
    kwargs = {}
    if trace:
        kwargs = dict(trace=True, trace_cores=list(range(N_CORES)))
    res = run_bass_kernel_spmd(nc, in_maps, core_ids=list(range(N_CORES)), **kwargs)
    parts = [res.results[i]["out"][:ROWS_PER_CORE] for i in range(N_CORES)]
    full = np.concatenate(parts, axis=0).reshape(B, L, 80)
    # quality_score has its own channel axis of size 1 in the reference; the
    # broadcast add makes the output (B, L, 80) either way.
    return full, res


def kernel(scores, pred_corners, W1, b1, W2, b2):
    full, _ = _run(
        np.asarray(scores),
        np.asarray(pred_corners),
        np.asarray(W1),
        np.asarray(b1),
        np.asarray(W2),
        np.asarray(b2),
    )
    return full

